# revision 9
# baseline (speedup 1.0000x reference)
"""Trainium2 Bass kernel for causal GQA attention (nn_Attention_83090437308676).

Full shapes: x [4096, 2048], 16 Q heads / 4 KV heads, d_head=128, fp32, causal,
rotary (interleaved pairs, rotary_dim=128), out = attn @ W_O + b_O.

Sharding: tensor-parallel over heads. Core c computes Q-heads {2c, 2c+1} and
KV-head c//2 (duplicated across the pair of cores sharing it), produces the
partial output z_h @ W_O_h summed over its 2 heads; the host sums the 8
partials (bf16 on the wire, fp32 accumulate) and adds b_O.

v5 design (v4 was 563-576us, PE 79.5% busy at ~1.8GHz effective):
- The softmax-denominator matmuls (2 per kt tile, same column count as the
  PV matmuls -- 17% of all PE cycles in v4) are gone from the PE. The Pool
  engine (otherwise idle) keeps a running elementwise sum of the exp tiles
  in SBUF; one tiny f32r ones-matmul per head per chunk turns it into the
  denominator row.
- One merged instruction stream per chunk: the previous chunk's
  normalization + output projection and the next chunk's Q/K/V projection
  are emitted as "weave" units distributed between the attention
  iterations, so the PE queue always holds ready matmuls while Act runs
  exp. (TRN2's PE drops to 1.2GHz for 3us after any idle gap -- continuity
  is worth more than any single-instruction win.)
- Bank-granular PSUM: scores double-buffered 2x[128,1024] (4 banks),
  PV accumulator 1x[128,1024] (2 banks), and a 2x[128,512] "halfbank" ring
  (2 banks) shared in strict program order by the denominator rows, the
  1/den broadcast, 16 output-projection tiles, and the per-head Q / K / V
  projection accumulators of the next chunk. No cross-phase ring coupling:
  v4's single 4-slot ring stalled every chunk start ~4.2us waiting for
  output-evac drains.
- zt is evacuated PSUM->SBUF (bf16, DVE) right at the chunk boundary so the
  single zt slot turns over immediately; z = ztf * (1/den) runs as an
  all-SBUF bf16 DVE multiply (4x mode).
- Output stores + V-transpose go on the Activation hwdge queue, x/weight
  loads on the sync queue, so load bursts never delay store drains.
"""

import numpy as np

SEQ = 4096
D_MODEL = 2048
D_HEAD = 128
N_HEADS = 16
N_KV = 4
N_CORES = 8
ROTARY_BASE = 10000.0
ATTN_SCALE = 11.313708498984761  # sqrt(d_head)

P = 128  # partitions
FD = 512  # matmul moving free dim / PSUM bank width (fp32)
FD2 = 2 * FD


def build_bass(seq=SEQ, d_model=D_MODEL, heads_per_core=2):
    """Emit the per-core Tile kernel. Same program for all cores (SPMD);
    per-core tensors differ only in data."""
    from contextlib import ExitStack

    import concourse.mybir as mybir
    import concourse.tile as tile
    from concourse import bacc
    import concourse.bass as cbass
    from concourse.bass import ds

    f32 = mybir.dt.float32
    f32r = mybir.dt.float32r
    bf16 = mybir.dt.bfloat16
    AF = mybir.ActivationFunctionType

    H = heads_per_core
    assert H == 2, "pairing assumes 2 heads per core"
    DM_TILES = d_model // P      # contraction tiles for projections
    QC = seq // FD               # 512-wide seq chunks
    MC = d_model // FD           # 512-wide output-model chunks
    KB = FD // P                 # 128-wide k blocks per chunk

    nc = bacc.Bacc("TRN2", target_bir_lowering=False, debug=False,
                   num_devices=N_CORES)

    xT = nc.dram_tensor("xT", (d_model, seq), bf16, kind="ExternalInput").ap()
    wq = nc.dram_tensor("wq", (H, d_model, D_HEAD), bf16, kind="ExternalInput").ap()
    wk = nc.dram_tensor("wk", (d_model, D_HEAD), bf16, kind="ExternalInput").ap()
    wv = nc.dram_tensor("wv", (d_model, D_HEAD), bf16, kind="ExternalInput").ap()
    wo = nc.dram_tensor("wo", (H, D_HEAD, d_model), bf16, kind="ExternalInput").ap()
    bq = nc.dram_tensor("bq", (64, H, 2), f32, kind="ExternalInput").ap()
    bk = nc.dram_tensor("bk", (64, 2), f32, kind="ExternalInput").ap()
    bv = nc.dram_tensor("bv", (P, 1), f32, kind="ExternalInput").ap()
    cos2 = nc.dram_tensor("cos2", (64, seq), f32, kind="ExternalInput").ap()
    sin2 = nc.dram_tensor("sin2", (64, seq), f32, kind="ExternalInput").ap()
    maskm = nc.dram_tensor("maskm", (P, P), bf16, kind="ExternalInput").ap()
    onesr = nc.dram_tensor("onesr", (1, P), f32r, kind="ExternalInput").ap()
    onescol = nc.dram_tensor("onescol", (P, 1), f32r, kind="ExternalInput").ap()
    out = nc.dram_tensor("out", (seq, d_model), bf16, kind="ExternalOutput").ap()

    with tile.TileContext(nc) as tc, ExitStack() as ctx:
        const = ctx.enter_context(tc.tile_pool(name="const", bufs=1))
        persist = ctx.enter_context(tc.tile_pool(name="persist", bufs=1))
        xt_pool = ctx.enter_context(tc.tile_pool(name="xt", bufs=32))
        qt_pool = ctx.enter_context(tc.tile_pool(name="qt", bufs=2))
        e_pool = ctx.enter_context(tc.tile_pool(name="e", bufs=4))
        sc_pool = ctx.enter_context(tc.tile_pool(name="sc", bufs=2))
        ps = ctx.enter_context(tc.tile_pool(name="ps", bufs=2, space="PSUM"))

        def stt(name):
            return ps.tile([P, FD2], f32, tag="st", bufs=2, name=name)

        def hbt(name):
            return ps.tile([P, FD], f32, tag="hb", bufs=2, name=name)

        # ---- constants / weights resident in SBUF ----
        wq_sb = const.tile([P, H, DM_TILES, D_HEAD], bf16, tag="wq")
        wk_sb = const.tile([P, DM_TILES, D_HEAD], bf16, tag="wk")
        wv_sb = const.tile([P, DM_TILES, D_HEAD], bf16, tag="wv")
        wq_r = wq.rearrange("h (t p) d -> p h t d", p=P)
        wk_r = wk.rearrange("(t p) d -> p t d", p=P)
        wv_r = wv.rearrange("(t p) d -> p t d", p=P)
        mask_sb = const.tile([P, P], bf16, tag="mask")
        nc.sync.dma_start(mask_sb[:], maskm)
        bq_sb = const.tile([64, H, 2], f32, tag="bq")
        nc.sync.dma_start(bq_sb[:], bq)
        bk_sb = const.tile([64, 2], f32, tag="bk")
        nc.sync.dma_start(bk_sb[:], bk)
        bv_sb = const.tile([P, 1], f32, tag="bv")
        nc.sync.dma_start(bv_sb[:], bv)
        onesr_sb = const.tile([1, P], f32r, tag="onesr")
        nc.sync.dma_start(onesr_sb[:], onesr)
        onescol_sb = const.tile([P, 1], f32r, tag="onescol")
        nc.sync.dma_start(onescol_sb[:], onescol)
        cos_sb = const.tile([64, seq], f32, tag="cos")
        sin_sb = const.tile([64, seq], f32, tag="sin")
        wo_sb = const.tile([P, H, d_model], bf16, tag="wo")
        # preload the Exp activation table off the critical path
        warm = const.tile([1, 2], f32, tag="warm")
        nc.scalar.activation(warm[0:1, 0:2], bq_sb[0:1, 0, 0:2], AF.Exp)

        # K^T (rotated) and V (natural [k, d]) for this core's KV head.
        kt_sb = persist.tile([P, seq], bf16, tag="kt")
        v_sb = persist.tile([P, seq // P, P], bf16, tag="v")

        # qb/kb staging for the rotaries (address reused every chunk; the
        # tag-ring serializes against the previous chunk's rotary reads,
        # which finished a full chunk ago).
        qb_lo = const.tile([64, FD2], f32, tag="qb_lo")
        qb_hi = const.tile([64, FD2], f32, tag="qb_hi")
        kb_lo = const.tile([64, FD], f32, tag="kb_lo")
        kb_hi = const.tile([64, FD], f32, tag="kb_hi")

        xts = {}   # chunk -> list of resident xT tiles
        qts = {}   # chunk -> rotated Q tile [P, H, FD] bf16

        # ---------------- rotary helpers ----------------
        def qb_copies(h, qp_ap):
            """Act: evacuate one head's Q projection (PSUM) into the qb
            staging halves with bias, freeing its halfbank slot."""
            nc.scalar.activation(qb_lo[:, ds(h * FD, FD)], qp_ap[0:64, :],
                                 AF.Identity, bias=bq_sb[:, h, 0:1])
            nc.scalar.activation(qb_hi[:, ds(h * FD, FD)], qp_ap[64:128, :],
                                 AF.Identity, bias=bq_sb[:, h, 1:2])

        def rotq(nqc):
            """DVE: both Q heads' rotary from qb staging into a fresh qt
            tile. Pair views with cos/sin broadcast across heads via a
            stride-0 free dim."""
            sl = ds(nqc * FD, FD)
            qt = qt_pool.tile([P, H, FD], bf16, tag="qt", name=f"qt_{nqc}")
            x1 = qb_lo[:].rearrange("p (h f) -> p h f", h=H)
            x2 = qb_hi[:].rearrange("p (h f) -> p h f", h=H)

            def cs_pair(src):
                ap = src[:, sl]
                return cbass.AP(ap.tensor, ap.offset,
                                [list(ap.ap[0]), [0, H], [1, FD]])

            cosp, sinp = cs_pair(cos_sb), cs_pair(sin_sb)
            t1 = sc_pool.tile([64, FD2], f32, tag="rot_t1", bufs=1)
            t2 = sc_pool.tile([64, FD2], f32, tag="rot_t2", bufs=1)
            t3 = sc_pool.tile([64, FD2], f32, tag="rot_t3", bufs=1)
            t4 = sc_pool.tile([64, FD2], f32, tag="rot_t4", bufs=1)
            pv = lambda t: t.rearrange("p (h f) -> p h f", h=H)
            nc.vector.tensor_mul(pv(t1), x1, cosp)
            nc.vector.tensor_mul(pv(t2), x2, sinp)
            nc.vector.tensor_mul(pv(t3), x1, sinp)
            nc.vector.tensor_mul(pv(t4), x2, cosp)
            nc.vector.tensor_sub(qt[0:64, :, :], pv(t1), pv(t2))
            nc.vector.tensor_add(qt[64:128, :, :], pv(t3), pv(t4))
            qts[nqc] = qt

        def kb_copies(kp_ap):
            nc.scalar.activation(kb_lo[:], kp_ap[0:64, :], AF.Identity,
                                 bias=bk_sb[:, 0:1])
            nc.scalar.activation(kb_hi[:], kp_ap[64:128, :], AF.Identity,
                                 bias=bk_sb[:, 1:2])

        def rotk(nqc):
            sl = ds(nqc * FD, FD)
            dst = kt_sb[:, sl]
            t1 = sc_pool.tile([64, FD], f32, tag="rk_t1", bufs=1)
            t2 = sc_pool.tile([64, FD], f32, tag="rk_t2", bufs=1)
            t3 = sc_pool.tile([64, FD], f32, tag="rk_t3", bufs=1)
            t4 = sc_pool.tile([64, FD], f32, tag="rk_t4", bufs=1)
            nc.vector.tensor_mul(t1[:], kb_lo[:], cos_sb[:, sl])
            nc.vector.tensor_mul(t2[:], kb_hi[:], sin_sb[:, sl])
            nc.vector.tensor_mul(t3[:], kb_lo[:], sin_sb[:, sl])
            nc.vector.tensor_mul(t4[:], kb_hi[:], cos_sb[:, sl])
            # rot1 = x1 cos - x2 sin ; rot2 = x1 sin + x2 cos
            nc.vector.tensor_sub(dst[0:64, :], t1[:], t2[:])
            nc.vector.tensor_add(dst[64:128, :], t3[:], t4[:])

        def v_finish(vp_ap, nqc):
            """Act: bias-add V (PSUM->SBUF bf16), then DMA-transpose to
            natural [k, d] on the Act hwdge queue."""
            vt = sc_pool.tile([P, FD], bf16, tag="vt", bufs=1, name=f"vt_{nqc}")
            nc.scalar.activation(vt[:], vp_ap, AF.Identity, bias=bv_sb[:, 0:1])
            nc.scalar.dma_start_transpose(v_sb[:, ds(nqc * KB, KB), :], vt[:])

        # ---------------- weave builders ----------------
        def build_proj(nqc):
            """Weave items projecting chunk nqc: 16 x-tile loads, per-head
            Q chains, K chain, V chain, with their rotaries/evacuations."""
            tiles = [xt_pool.tile([P, FD], bf16, tag="xt", name=f"xt_{nqc}_{t}")
                     for t in range(DM_TILES)]
            xts[nqc] = tiles
            items = []
            for t in range(DM_TILES):
                def ld(t=t):
                    # alternate hwdge queues to halve the load-burst latency
                    eng = nc.sync if t % 2 == 0 else nc.scalar
                    eng.dma_start(tiles[t][:],
                                  xT[ds(t * P, P), ds(nqc * FD, FD)])
                items.append((1, ld))

            def qp(h):
                def fn(h=h):
                    qph = hbt(f"qp{h}_{nqc}")
                    for t in range(DM_TILES):
                        nc.tensor.matmul(qph[:], wq_sb[:, h, t, :], tiles[t][:],
                                         start=(t == 0),
                                         stop=(t == DM_TILES - 1))
                    qb_copies(h, qph)
                return fn

            items.append((DM_TILES, qp(0)))
            items.append((DM_TILES, qp(1)))
            items.append((0, lambda: rotq(nqc)))

            def kp():
                kph = hbt(f"kp_{nqc}")
                for t in range(DM_TILES):
                    nc.tensor.matmul(kph[:], wk_sb[:, t, :], tiles[t][:],
                                     start=(t == 0), stop=(t == DM_TILES - 1))
                kb_copies(kph)

            items.append((DM_TILES, kp))
            items.append((0, lambda: rotk(nqc)))

            def vp():
                vph = hbt(f"vp_{nqc}")
                for t in range(DM_TILES):
                    nc.tensor.matmul(vph[:], wv_sb[:, t, :], tiles[t][:],
                                     start=(t == 0), stop=(t == DM_TILES - 1))
                v_finish(vph, nqc)
                xts.pop(nqc)

            items.append((DM_TILES, vp))
            return items

        def build_norm_op(pqc, st):
            """Weave items finishing chunk pqc: denominator matmuls, softmax
            normalization (reciprocal, 1/den broadcast, z multiply) and the
            16 output projection tiles with their evacuations + stores.
            Returned as (head, tail): head runs at the next chunk's start,
            tail (bcast + op tiles) after its projection chains so the PE
            never waits on the DVE normalization pipeline."""
            zt2 = st["zt2"]
            esum = st["esum"]
            box = {}

            def denmm():
                # ones^T @ esum fills the iteration-0 exp bubble on the PE
                den0 = hbt(f"den0_{pqc}")
                den1 = hbt(f"den1_{pqc}")
                nc.tensor.matmul(den0[0:1, :], onescol_sb[:, 0:1],
                                 esum[:, 0:FD], start=True, stop=True)
                nc.tensor.matmul(den1[0:1, :], onescol_sb[:, 0:1],
                                 esum[:, FD:FD2], start=True, stop=True)
                box["den"] = (den0, den1)

            def pre():
                den0, den1 = box["den"]
                # DVE: free the zt slot (bf16 copy out) + reciprocals.
                ztf = sc_pool.tile([P, FD2], bf16, tag="ztf", bufs=2,
                                   name=f"ztf_{pqc}")
                nc.vector.tensor_copy(ztf[:], zt2[:])
                rf = sc_pool.tile([1, FD2], f32, tag="rf", bufs=1,
                                  name=f"rf_{pqc}")
                nc.vector.reciprocal_approx_fast(rf[0:1, 0:FD], den0[0:1, :])
                nc.vector.reciprocal_approx_fast(rf[0:1, FD:FD2], den1[0:1, :])
                rr = sc_pool.tile([1, FD2], f32r, tag="rr", bufs=1,
                                  name=f"rr_{pqc}")
                nc.vector.tensor_scalar_mul(rr[:], rf[:], 1.0)
                box["ztf"] = ztf
                box["rr"] = rr

            def bcast():
                # PE: broadcast 1/den across partitions; DVE: bf16 copy out
                # + the normalization multiply.
                b0 = hbt(f"bc0_{pqc}")
                b1 = hbt(f"bc1_{pqc}")
                nc.tensor.matmul(b0[:], onesr_sb[:], box["rr"][0:1, 0:FD],
                                 start=True, stop=True)
                nc.tensor.matmul(b1[:], onesr_sb[:], box["rr"][0:1, FD:FD2],
                                 start=True, stop=True)
                rdenf = sc_pool.tile([P, FD2], bf16, tag="rdenf", bufs=2,
                                     name=f"rdenf_{pqc}")
                nc.vector.tensor_copy(rdenf[:, 0:FD], b0[:])
                nc.vector.tensor_copy(rdenf[:, FD:FD2], b1[:])
                z2 = sc_pool.tile([P, FD2], bf16, tag="z", bufs=2,
                                  name=f"z2_{pqc}")
                nc.vector.tensor_mul(z2[:], box["ztf"][:], rdenf[:])
                box["z2"] = z2

            head = [(2, denmm), (2, pre)]
            tail = [(2, bcast)]
            for j in range(KB * MC):
                sub, mc = j % KB, j // KB

                def op(sub=sub, mc=mc):
                    opt = hbt(f"op_{pqc}_{sub}_{mc}")
                    for h in range(H):
                        nc.tensor.matmul(opt[:],
                                         box["z2"][:, ds(h * FD + sub * P, P)],
                                         wo_sb[:, h, ds(mc * FD, FD)],
                                         start=(h == 0), stop=(h == H - 1))
                    ot = sc_pool.tile([P, FD], bf16, tag="ot", bufs=4,
                                      name=f"ot_{pqc}_{sub}_{mc}")
                    nc.vector.tensor_copy(ot[:], opt[:])
                    nc.scalar.dma_start(
                        out[ds(pqc * FD + sub * P, P), ds(mc * FD, FD)], ot[:])

                tail.append((2, op))
            return head, tail

        # ---------------- attention ----------------
        def attention(qc, qt, weave):
            """Causal attention for q chunk qc with weave units distributed
            between the kt iterations. Returns the zt/den handles for the
            deferred normalization."""
            KT = 4 * qc + 4
            zt2 = ps.tile([P, FD2], f32, tag="zt", bufs=1, name=f"zt_{qc}")
            esum = sc_pool.tile([P, FD2], f32r, tag="esum", bufs=2,
                                name=f"esum_{qc}")
            esv = esum.rearrange("p (h f) -> p h f", h=H)
            wtot = sum(w for w, _ in weave)
            span = max(KT - 1, 1)  # drain fully one iter early: the DVE
            # evac backlog then clears before the next chunk's den matmuls
            widx = 0
            cum = 0
            pend = None
            for kt in range(KT):
                target = (wtot * (kt + 1) + span - 1) // span
                while widx < len(weave) and cum < target:
                    w, fn = weave[widx]
                    fn()
                    cum += w
                    widx += 1
                o = max(0, kt * P - qc * FD)
                n = FD - o
                st2 = stt(f"st_{qc}_{kt}")
                for h in range(H):
                    nc.tensor.matmul(st2[:, ds(h * FD + o, n)],
                                     kt_sb[:, ds(kt * P, P)], qt[:, h, o:FD],
                                     start=True, stop=True)
                e2 = e_pool.tile([P, FD2], bf16, tag="e", name=f"e_{qc}_{kt}")
                ev = e2.rearrange("p (h f) -> p h f", h=H)
                sv = st2.rearrange("p (h f) -> p h f", h=H)
                nc.scalar.activation(ev[:, :, o:FD], sv[:, :, o:FD], AF.Exp,
                                     scale=1.0 / ATTN_SCALE)
                if kt >= 4 * qc:  # diagonal 128-block: causal mask inside
                    # on Pool (idle otherwise): keeps the DVE free for the
                    # rotary/evac stream, which would delay PV via the mask
                    for h in range(H):
                        nc.gpsimd.tensor_mul(e2[:, ds(h * FD + o, P)],
                                             e2[:, ds(h * FD + o, P)],
                                             mask_sb[:])
                # Pool: running sum of exp tiles (the softmax denominator
                # before the cross-partition reduction)
                if kt == 0:
                    nc.gpsimd.tensor_copy(esv[:, :, :], ev[:, :, :])
                else:
                    nc.gpsimd.tensor_add(esv[:, :, o:FD], esv[:, :, o:FD],
                                         ev[:, :, o:FD])
                if pend is not None:
                    pkt, pe2, po, pn = pend
                    acc = dict(start=(pkt == 0), stop=(pkt == KT - 1))
                    for h in range(H):
                        nc.tensor.matmul(zt2[:, ds(h * FD + po, pn)],
                                         v_sb[:, pkt, :],
                                         pe2[:, ds(h * FD + po, pn)], **acc)
                pend = (kt, e2, o, n)
            pkt, pe2, po, pn = pend
            acc = dict(start=(pkt == 0), stop=(pkt == KT - 1))
            for h in range(H):
                nc.tensor.matmul(zt2[:, ds(h * FD + po, pn)], v_sb[:, pkt, :],
                                 pe2[:, ds(h * FD + po, pn)], **acc)
            while widx < len(weave):
                weave[widx][1]()
                widx += 1
            return {"zt2": zt2, "esum": esum}

        # ---------------- preamble: chunk 0 projection ----------------
        # DMA-bound cold start: all four projection chains t-major with
        # just-in-time loads, borrowing the (idle) score slots for the
        # chunk-0 Q and K|V accumulators.
        qp_st = stt("qp_c0")
        kv_st = stt("kv_c0")
        kp0, vp0 = kv_st[:, 0:FD], kv_st[:, FD:FD2]
        nc.sync.dma_start(cos_sb[:, 0:FD], cos2[:, 0:FD])
        nc.sync.dma_start(sin_sb[:, 0:FD], sin2[:, 0:FD])
        tiles0 = [xt_pool.tile([P, FD], bf16, tag="xt", name=f"xt_0_{t}")
                  for t in range(DM_TILES)]
        xts[0] = tiles0
        for t in range(DM_TILES):
            nc.sync.dma_start(tiles0[t][:], xT[ds(t * P, P), ds(0, FD)])
            nc.sync.dma_start(wk_sb[:, t, :], wk_r[:, t, :])
            nc.sync.dma_start(wq_sb[:, :, t, :], wq_r[:, :, t, :])
            nc.sync.dma_start(wv_sb[:, t, :], wv_r[:, t, :])
            mm = dict(start=(t == 0), stop=(t == DM_TILES - 1))
            nc.tensor.matmul(kp0, wk_sb[:, t, :], tiles0[t][:], **mm)
            for h in range(H):
                nc.tensor.matmul(qp_st[:, ds(h * FD, FD)],
                                 wq_sb[:, h, t, :], tiles0[t][:], **mm)
            nc.tensor.matmul(vp0, wv_sb[:, t, :], tiles0[t][:], **mm)
        nc.sync.dma_start(cos_sb[:, FD:seq], cos2[:, FD:seq])
        nc.sync.dma_start(sin_sb[:, FD:seq], sin2[:, FD:seq])
        nc.sync.dma_start(wo_sb[:], wo.rearrange("h p m -> p h m"))
        kb_copies(kp0)
        rotk(0)
        qb_copies(0, qp_st[:, 0:FD])
        qb_copies(1, qp_st[:, FD:FD2])
        rotq(0)
        v_finish(vp0, 0)
        xts.pop(0)

        # ---------------- main loop ----------------
        # weave order per chunk qc: [den+recips of qc-1] [x loads qc+1]
        # [Q/K/V chains qc+1] [1/den broadcast + output tiles of qc-1].
        # This order IS the halfbank ring order, so every allocation's
        # wait is on work already finished a few iterations earlier.
        stages = None
        for qc in range(QC):
            items = []
            tail = []
            if stages is not None:
                head, tail = build_norm_op(qc - 1, stages)
                items += head
            if qc + 1 < QC:
                proj = build_proj(qc + 1)
                items += proj[:DM_TILES]    # loads
                items += proj[DM_TILES:]    # chains + rotaries
            items += tail
            stages = attention(qc, qts.pop(qc), items)
        head, tail = build_norm_op(QC - 1, stages)
        for w, fn in head + tail:
            fn()
    nc.compile()
    return nc


_PERM = None


def _perm():
    global _PERM
    if _PERM is None:
        _PERM = np.concatenate([np.arange(0, D_HEAD, 2), np.arange(1, D_HEAD, 2)])
    return _PERM


def host_inputs(x, W_Q, W_K, W_V, W_O, b_Q, b_K, b_V, core,
                heads_per_core=2):
    """Build the per-core input map (numpy, named as in build_bass)."""
    import ml_dtypes

    bf16 = ml_dtypes.bfloat16
    seq = x.shape[0]
    perm = _perm()
    h0 = core * heads_per_core
    kv = h0 // (N_HEADS // N_KV)
    pairs = D_HEAD // 2
    freqs = 1.0 / ROTARY_BASE ** (np.arange(pairs, dtype=np.float64) / pairs)
    ang = np.outer(np.arange(seq), freqs)  # [seq, 64]
    cos = np.cos(ang).T.astype(np.float32)  # [64, seq]
    sin = np.sin(ang).T.astype(np.float32)
    return {
        "xT": np.ascontiguousarray(np.asarray(x).T.astype(bf16)),
        "wq": np.ascontiguousarray(
            W_Q[h0:h0 + heads_per_core][:, :, perm].astype(bf16)),
        "wk": np.ascontiguousarray(W_K[kv][:, perm].astype(bf16)),
        "wv": np.ascontiguousarray(W_V[kv].astype(bf16)),
        "wo": np.ascontiguousarray(W_O[h0:h0 + heads_per_core].astype(bf16)),
        "bq": np.ascontiguousarray(
            b_Q[h0:h0 + heads_per_core][:, perm]
            .reshape(heads_per_core, 2, 64).transpose(2, 0, 1)
            .astype(np.float32)),
        "bk": np.ascontiguousarray(b_K[kv][perm].reshape(2, 64).T
                                   .astype(np.float32)),
        "bv": np.ascontiguousarray(np.asarray(b_V[kv], np.float32)[:, None]),
        "cos2": cos,
        "sin2": sin,
        "maskm": np.triu(np.ones((P, P), dtype=np.float32)).astype(bf16),
        "onesr": np.ones((1, P), dtype=np.float32),
        "onescol": np.ones((P, 1), dtype=np.float32),
    }


_NC_CACHE = {}


def kernel(x, W_Q, W_K, W_V, W_O, b_Q, b_K, b_V, b_O):
    import sys
    if "/opt/trn_rl_repo" not in sys.path:
        sys.path.insert(0, "/opt/trn_rl_repo")
    from concourse import bass_utils

    x = np.asarray(x, dtype=np.float32)
    key = (x.shape[0], x.shape[1])
    if key not in _NC_CACHE:
        _NC_CACHE[key] = build_bass(seq=x.shape[0], d_model=x.shape[1])
    nc = _NC_CACHE[key]

    in_maps = [
        host_inputs(x, np.asarray(W_Q, np.float32), np.asarray(W_K, np.float32),
                    np.asarray(W_V, np.float32), np.asarray(W_O, np.float32),
                    np.asarray(b_Q, np.float32), np.asarray(b_K, np.float32),
                    np.asarray(b_V, np.float32), core)
        for core in range(N_CORES)
    ]
    res = bass_utils.run_bass_kernel_spmd(nc, in_maps, core_ids=list(range(N_CORES)))
    total = np.zeros((x.shape[0], x.shape[1]), dtype=np.float32)
    for r in res.results:
        total += np.asarray(r["out"], dtype=np.float32)
    total += np.asarray(b_O, np.float32)[None, :]
    return total
